# revision 1
# baseline (speedup 1.0000x reference)
"""GraphSAGE 3-layer kernel for 8 trn2 NeuronCores.

Sharding: dst-node parallel. Each core owns 6250 dst nodes (padded 6272).
Per layer: gather x[src] rows per edge (indirect DMA), segment-sum via
one-hot matmul into PSUM (feature-major), scale by 1/deg, two GEMMs
(Wl@mean + Wr@x), BN stats via AllReduce, BN+ReLU, transpose own slice to
node-major, AllGather into the next layer's gather table.
"""
import numpy as np

N = 50000
E = 800000
IN_DIM = 128
HID = 128
OUT_DIM = 64
BN_EPS = 1e-5
NC_ = 8
NPC = 6250          # real nodes per core
PADC = 6272         # padded nodes per core (49*128)
R = NC_ * PADC      # remapped table rows (50176)
NB = PADC // 128    # dst blocks per core (49)
P = 128


def _remap(n):
    return (n // NPC) * PADC + (n % NPC)


def _preprocess(x, edge_index):
    src = edge_index[0].astype(np.int64)
    dst = edge_index[1].astype(np.int64)
    deg = np.bincount(dst, minlength=N).astype(np.float32)
    recip = 1.0 / np.maximum(deg, 1.0)
    src_r = _remap(src)

    per_core = []
    counts = np.zeros((NC_, NB), dtype=np.int64)
    for c in range(NC_):
        m = (dst // NPC) == c
        s = src_r[m]
        dl = (dst[m] - c * NPC)
        order = np.argsort(dl, kind="stable")
        s, dl = s[order], dl[order]
        blk = dl // 128
        for b in range(NB):
            counts[c, b] = np.count_nonzero(blk == b)
        per_core.append((s, dl, blk))

    nT = np.maximum(1, np.ceil(counts / 128.0).astype(np.int64).max(axis=0))
    tOff = np.concatenate([[0], np.cumsum(nT)]).astype(np.int64)
    totT = int(tOff[-1])

    src_idx = np.full((NC_, P, totT), NPC, dtype=np.int32)  # pad -> zero row 6250
    dstloc = np.zeros((NC_, P, totT), dtype=np.float32)
    recip_b = np.zeros((NC_, P, PADC), dtype=np.float32)
    for c in range(NC_):
        s, dl, blk = per_core[c]
        for b in range(NB):
            sel = blk == b
            sb, db = s[sel], (dl[sel] % 128)
            k = len(sb)
            slots = np.arange(k)
            pp, tt = slots % P, tOff[b] + slots // P
            src_idx[c, pp, tt] = sb
            dstloc[c, pp, tt] = db.astype(np.float32)
        recip_b[c, :, :NPC] = recip[c * NPC:(c + 1) * NPC][None, :]

    x_table = np.zeros((R, IN_DIM), dtype=np.float32)
    for c in range(NC_):
        x_table[c * PADC:c * PADC + NPC] = x[c * NPC:(c + 1) * NPC]
    xT_own = np.zeros((NC_, P, PADC), dtype=np.float32)
    for c in range(NC_):
        xT_own[c, :, :NPC] = x[c * NPC:(c + 1) * NPC].T
    return src_idx, dstloc, recip_b, x_table, xT_own, nT, tOff, totT


def _build(nT, tOff, totT):
    import concourse.bass as bass
    import concourse.bacc as bacc
    import concourse.tile as tile
    from concourse import mybir
    from concourse.masks import make_identity

    f32 = mybir.dt.float32
    nc = bacc.Bacc("TRN2")
    t_x = nc.dram_tensor("x_table", [R, IN_DIM], f32, kind="ExternalInput")
    t_idx = nc.dram_tensor("src_idx", [P, totT], mybir.dt.int32, kind="ExternalInput")
    t_dl = nc.dram_tensor("dstloc", [P, totT], f32, kind="ExternalInput")
    t_rc = nc.dram_tensor("recip", [P, PADC], f32, kind="ExternalInput")
    t_xT = nc.dram_tensor("xT_own", [P, PADC], f32, kind="ExternalInput")
    t_w = {}
    for l, fo in ((1, HID), (2, HID), (3, OUT_DIM)):
        t_w[f"wl{l}"] = nc.dram_tensor(f"wl{l}", [P, fo], f32, kind="ExternalInput")
        t_w[f"wr{l}"] = nc.dram_tensor(f"wr{l}", [P, fo], f32, kind="ExternalInput")
    t_bn = nc.dram_tensor("bn", [P, 4], f32, kind="ExternalInput")  # g1 be1 g2 be2
    t_out = nc.dram_tensor("out", [PADC, OUT_DIM], f32, kind="ExternalOutput")

    h_own = [nc.dram_tensor(f"h_own{l}", [PADC, HID], f32, kind="Internal") for l in (1, 2)]
    h_tab = [nc.dram_tensor(f"h_tab{l}", [R, HID], f32, kind="Internal", addr_space="Shared") for l in (1, 2)]
    st_in = [nc.dram_tensor(f"st_in{l}", [P, 2], f32, kind="Internal") for l in (1, 2)]
    st_out = [nc.dram_tensor(f"st_out{l}", [P, 2], f32, kind="Internal", addr_space="Shared") for l in (1, 2)]
    RG = [list(range(NC_))]

    with tile.TileContext(nc) as tc:
        import contextlib
        with contextlib.ExitStack() as ctx:
            singles = ctx.enter_context(tc.tile_pool(name="singles", bufs=1))
            gp = ctx.enter_context(tc.tile_pool(name="g", bufs=8))
            sp = ctx.enter_context(tc.tile_pool(name="s", bufs=8))
            pseg = ctx.enter_context(tc.tile_pool(name="pseg", bufs=4, space="PSUM"))
            pgem = ctx.enter_context(tc.tile_pool(name="pgem", bufs=2, space="PSUM"))
            ptr = ctx.enter_context(tc.tile_pool(name="ptr", bufs=2, space="PSUM"))
            trp = ctx.enter_context(tc.tile_pool(name="trs", bufs=4))

            idx_sb = singles.tile([P, totT], mybir.dt.int32)
            dl_sb = singles.tile([P, totT], f32)
            rc_sb = singles.tile([P, PADC], f32)
            xT_sb = singles.tile([P, PADC], f32)
            nc.sync.dma_start(out=idx_sb[:], in_=t_idx[:])
            nc.sync.dma_start(out=dl_sb[:], in_=t_dl[:])
            nc.sync.dma_start(out=rc_sb[:], in_=t_rc[:])
            nc.sync.dma_start(out=xT_sb[:], in_=t_xT[:])
            w_sb = {}
            for k, t in t_w.items():
                w_sb[k] = singles.tile([P, t.shape[1]], f32, name=f"w_{k}", tag=f"w_{k}")
                nc.sync.dma_start(out=w_sb[k][:], in_=t[:])
            bn_sb = singles.tile([P, 4], f32)
            nc.sync.dma_start(out=bn_sb[:], in_=t_bn[:])
            ident = singles.tile([P, P], f32)
            make_identity(nc, ident[:])
            iota_i = singles.tile([P, P], mybir.dt.int32)
            nc.gpsimd.iota(iota_i[:], pattern=[[1, P]], base=0, channel_multiplier=0)
            iota_f = singles.tile([P, P], f32)
            nc.vector.tensor_copy(out=iota_f[:], in_=iota_i[:])
            eps_sb = singles.tile([P, 1], f32)
            nc.vector.memset(eps_sb[:], BN_EPS)

            agg = singles.tile([P, PADC], f32)
            hpre = singles.tile([P, PADC], f32)
            hbn = singles.tile([P, PADC], f32)
            sq = singles.tile([P, PADC], f32)
            nc.vector.memset(hpre[:], 0.0)

            chunks = [(i * 512, 512) for i in range(PADC // 512)]
            if PADC % 512:
                chunks.append(((PADC // 512) * 512, PADC % 512))

            for layer in (1, 2, 3):
                table = t_x if layer == 1 else h_tab[layer - 2]
                xTc = xT_sb if layer == 1 else hbn
                fo = OUT_DIM if layer == 3 else HID
                # --- segment sum: aggT[f, dst] ---
                for b in range(NB):
                    ps = pseg.tile([P, P], f32)
                    n_t = int(nT[b])
                    for ti in range(n_t):
                        t_g = int(tOff[b]) + ti
                        g = gp.tile([P, P], f32, tag="g")
                        nc.gpsimd.indirect_dma_start(
                            out=g[:], out_offset=None, in_=table[:],
                            in_offset=bass.IndirectOffsetOnAxis(ap=idx_sb[:, t_g:t_g + 1], axis=0),
                        )
                        s_t = sp.tile([P, P], f32, tag="s")
                        nc.vector.tensor_tensor(
                            out=s_t[:], in0=dl_sb[:, t_g:t_g + 1].to_broadcast([P, P]),
                            in1=iota_f[:], op=mybir.AluOpType.is_equal)
                        nc.tensor.matmul(out=ps[:], lhsT=g[:], rhs=s_t[:],
                                         start=(ti == 0), stop=(ti == n_t - 1))
                    nc.vector.tensor_tensor(out=agg[:, b * P:(b + 1) * P], in0=ps[:],
                                            in1=rc_sb[:, b * P:(b + 1) * P], op=mybir.AluOpType.mult)
                # --- GEMMs ---
                for off, w in chunks:
                    pg = pgem.tile([P, 512], f32, tag="pg")
                    nc.tensor.matmul(out=pg[:fo, :w], lhsT=w_sb[f"wl{layer}"][:],
                                     rhs=agg[:, off:off + w], start=True, stop=False)
                    nc.tensor.matmul(out=pg[:fo, :w], lhsT=w_sb[f"wr{layer}"][:],
                                     rhs=xTc[:, off:off + w], start=False, stop=True)
                    nc.vector.tensor_copy(out=hpre[:fo, off:off + w], in_=pg[:fo, :w])
                if layer < 3:
                    li = layer - 1
                    s1 = trp.tile([P, 1], f32, tag="st")
                    nc.vector.tensor_reduce(out=s1[:], in_=hpre[:], axis=mybir.AxisListType.X,
                                            op=mybir.AluOpType.add)
                    nc.vector.tensor_tensor(out=sq[:], in0=hpre[:], in1=hpre[:], op=mybir.AluOpType.mult)
                    s2 = trp.tile([P, 1], f32, tag="st")
                    nc.vector.tensor_reduce(out=s2[:], in_=sq[:], axis=mybir.AxisListType.X,
                                            op=mybir.AluOpType.add)
                    stt = trp.tile([P, 2], f32, tag="st2")
                    nc.vector.tensor_copy(out=stt[:, 0:1], in_=s1[:])
                    nc.vector.tensor_copy(out=stt[:, 1:2], in_=s2[:])
                    nc.sync.dma_start(out=st_in[li][:], in_=stt[:])
                    nc.gpsimd.collective_compute(
                        "AllReduce", mybir.AluOpType.add, replica_groups=RG,
                        ins=[st_in[li][:]], outs=[st_out[li][:]])
                    str_ = trp.tile([P, 2], f32, tag="st2")
                    nc.sync.dma_start(out=str_[:], in_=st_out[li][:])
                    mu = trp.tile([P, 1], f32, tag="st")
                    nc.scalar.mul(out=mu[:], in_=str_[:, 0:1], mul=1.0 / N)
                    ex2 = trp.tile([P, 1], f32, tag="st")
                    nc.scalar.mul(out=ex2[:], in_=str_[:, 1:2], mul=1.0 / N)
                    var = trp.tile([P, 1], f32, tag="st")
                    nc.vector.tensor_tensor(out=var[:], in0=mu[:], in1=mu[:], op=mybir.AluOpType.mult)
                    nc.vector.tensor_tensor(out=var[:], in0=ex2[:], in1=var[:], op=mybir.AluOpType.subtract)
                    rs = trp.tile([P, 1], f32, tag="st")
                    nc.scalar.activation(out=rs[:], in_=var[:], func=mybir.ActivationFunctionType.Sqrt,
                                         bias=eps_sb[:], scale=1.0, alpha=0.0)
                    nc.vector.reciprocal(out=rs[:], in_=rs[:])
                    a_t = trp.tile([P, 1], f32, tag="st")
                    nc.vector.tensor_tensor(out=a_t[:], in0=rs[:], in1=bn_sb[:, 2 * li:2 * li + 1],
                                            op=mybir.AluOpType.mult)
                    bi = trp.tile([P, 1], f32, tag="st")
                    nc.vector.tensor_tensor(out=bi[:], in0=mu[:], in1=a_t[:], op=mybir.AluOpType.mult)
                    nc.vector.tensor_tensor(out=bi[:], in0=bn_sb[:, 2 * li + 1:2 * li + 2], in1=bi[:],
                                            op=mybir.AluOpType.subtract)
                    nc.vector.tensor_scalar(out=hbn[:], in0=hpre[:], scalar1=a_t[:],
                                            scalar2=bi[:], op0=mybir.AluOpType.mult,
                                            op1=mybir.AluOpType.add)
                    nc.vector.tensor_scalar_max(out=hbn[:], in0=hbn[:], scalar1=0.0)
                    nc.vector.memset(hbn[:, NPC:PADC], 0.0)
                    for k in range(NB):
                        pt = ptr.tile([P, P], f32, tag="pt")
                        nc.tensor.transpose(out=pt[:], in_=hbn[:, k * P:(k + 1) * P], identity=ident[:])
                        ts_ = trp.tile([P, P], f32, tag="ts")
                        nc.vector.tensor_copy(out=ts_[:], in_=pt[:])
                        nc.sync.dma_start(out=h_own[li][k * P:(k + 1) * P, :], in_=ts_[:])
                    nc.gpsimd.collective_compute(
                        "AllGather", mybir.AluOpType.bypass, replica_groups=RG,
                        ins=[h_own[li][:]], outs=[h_tab[li][:]])
                else:
                    for k in range(NB):
                        pt = ptr.tile([P, P], f32, tag="pt")
                        nc.tensor.transpose(out=pt[:], in_=hpre[:, k * P:(k + 1) * P], identity=ident[:])
                        ts_ = trp.tile([P, P], f32, tag="ts")
                        nc.vector.tensor_copy(out=ts_[:], in_=pt[:])
                        nc.sync.dma_start(out=t_out[k * P:(k + 1) * P, :], in_=ts_[:, :OUT_DIM])
    nc.compile()
    return nc


def kernel(**inputs):
    import os
    os.environ.setdefault("BASS_NEVER_TRACE", "1")
    from concourse.bass_utils import run_bass_kernel_spmd

    x = np.asarray(inputs["x"], dtype=np.float32)
    ei = np.asarray(inputs["edge_index"])
    src_idx, dstloc, recip_b, x_table, xT_own, nT, tOff, totT = _preprocess(x, ei)
    nc = _build(nT, tOff, totT)

    bn = np.stack([np.asarray(inputs["g1"]), np.asarray(inputs["be1"]),
                   np.asarray(inputs["g2"]), np.asarray(inputs["be2"])], axis=1).astype(np.float32)
    wm = {}
    for l, (wl, wr) in {1: ("Wl1", "Wr1"), 2: ("Wl2", "Wr2"), 3: ("Wl3", "Wr3")}.items():
        wm[f"wl{l}"] = np.ascontiguousarray(np.asarray(inputs[wl], dtype=np.float32).T)
        wm[f"wr{l}"] = np.ascontiguousarray(np.asarray(inputs[wr], dtype=np.float32).T)

    in_maps = []
    for c in range(NC_):
        m = {"x_table": x_table, "src_idx": src_idx[c], "dstloc": dstloc[c],
             "recip": recip_b[c], "xT_own": xT_own[c], "bn": bn}
        m.update(wm)
        in_maps.append(m)
    res = run_bass_kernel_spmd(nc, in_maps, core_ids=list(range(NC_)))
    out = np.concatenate([res.results[c]["out"][:NPC] for c in range(NC_)], axis=0)
    return out.astype(np.float32)

